# revision 73
# baseline (speedup 1.0000x reference)
"""Trainium2 Bass kernel for Ernie4.5 attention (B=1, S=2048, HID=4096, H=32,
KVH=8, D=128), tensor-parallel over heads across 8 NeuronCores.

Core i owns q-heads 4i..4i+3, kv-head i, and wo rows [512*i, 512*(i+1)).
Each core computes its partial output [S, HID] in bf16; the host sums the 8
partials in fp32.

Per-core pipeline (all in transposed [feature, seq] layouts so no on-chip
transposition of activations is ever needed):
  1. qT/kT/vT = (w.T @ hsT-chunks) with weights stationary   -> [D, S] tiles
  2. RoPE on qT/kT via stream_shuffle (even/odd partition swap) + host tables
  3. chunk-outer attention: for each 512-seq chunk j, for each head:
     scoresT[sk,sq] = kT.T @ qT ; probsT = exp(scale*scoresT) with causal
     masking via 0/1 diag masks; outT[d,sq] += v.T @ probsT in PSUM; the
     prob tiles are folded by a binary tree of in-place bf16 adds on the
     vector engine down to ONE rowsum matmul per (head, chunk) (vs one per
     sk-block naively — each rowsum costs a full 512-col PE stream no
     matter how thin its output); normalize via fast-reciprocal of the
     rowsums read from PSUM + gpsimd partition_broadcast + vector mul
  4. wo projection of chunk j immediately after its 4 heads finish (weights
     SBUF-resident), emitted at low scheduler priority so its matmuls and
     evictions fill engine gaps during chunk j+1's attention; bf16 partials
     DMA'd out on both queues while attention proceeds.
"""

import os
import sys
from contextlib import ExitStack

import numpy as np

for _p in ("/opt/trn_rl_repo",):
    if os.path.isdir(_p) and _p not in sys.path:
        sys.path.append(_p)

import ml_dtypes

import concourse.bass as bass
import concourse.mybir as mybir
import concourse.tile as tile
from concourse import bacc
from concourse.bass_utils import run_bass_kernel_spmd
from concourse.masks import make_identity

P = 128
B, S, HID, H, KVH, D = 1, 2048, 4096, 32, 8, 128
NCORES = 8
HL = H // NCORES          # 4 local q heads
NKT = HID // P            # 32 contraction tiles
NSQ = S // P              # 16 seq blocks
CW = 512                  # seq chunk width
NCH = S // CW             # 4 seq chunks
KP = 4                    # hsT k-tiles packed per DMA
WOC = 512                 # wo output chunk width
NHC = HID // WOC          # 8 wo output chunks
NCB = HL + 2              # 6 projection column blocks (4 q heads, k, v)
SCALE = float(D) ** -0.5
BASE = 10000.0

F32 = mybir.dt.float32
BF16 = mybir.dt.bfloat16
SWAP_MASK = [i ^ 1 for i in range(32)]

LAST_RESULT = None


def _build(act_dt=BF16, table_dt=F32):
    """Emit the SPMD per-core program. act_dt = matmul operand dtype."""
    nc = bacc.Bacc("TRN2", target_bir_lowering=False, debug=False)

    # hsT host-packed as [g, piece, p, kp*s] so each pack DMA is one fully
    # contiguous 512KB block (4KB per partition line) instead of 1KB lines
    hsT_d = nc.dram_tensor(
        "hsT", [NKT // KP, NCH, P, KP * CW], act_dt, kind="ExternalInput").ap()
    wqkv_d = nc.dram_tensor("wqkv", [NCB, P, NKT * P], act_dt, kind="ExternalInput").ap()
    wo_d = nc.dram_tensor("wo", [HL, P, NHC, WOC], act_dt, kind="ExternalInput").ap()
    cosT_d = nc.dram_tensor("cosT", [P, S], table_dt, kind="ExternalInput").ap()
    ssinT_d = nc.dram_tensor("ssinT", [P, S], table_dt, kind="ExternalInput").ap()
    dmask_d = nc.dram_tensor("dmask", [P, CW // P, CW], act_dt, kind="ExternalInput").ap()
    out_d = nc.dram_tensor("out", [S, HID], act_dt, kind="ExternalOutput").ap()

    with tile.TileContext(nc) as tc, ExitStack() as ctx:
        const = ctx.enter_context(tc.tile_pool(name="const", bufs=1))
        wpool = ctx.enter_context(tc.tile_pool(name="wpool", bufs=1))
        tabs = ctx.enter_context(tc.tile_pool(name="tabs", bufs=1))
        res = ctx.enter_context(tc.tile_pool(name="res", bufs=1))
        outc = ctx.enter_context(tc.tile_pool(name="outc", bufs=2))
        hst = ctx.enter_context(tc.tile_pool(name="hst", bufs=NKT // KP + 1))
        evq = ctx.enter_context(tc.tile_pool(name="evq", bufs=2))
        rope = ctx.enter_context(tc.tile_pool(name="rope", bufs=2))
        vtmp = ctx.enter_context(tc.tile_pool(name="vtmp", bufs=1))
        probs = ctx.enter_context(tc.tile_pool(name="probs", bufs=12))
        norm = ctx.enter_context(tc.tile_pool(name="norm", bufs=1))
        normb = ctx.enter_context(tc.tile_pool(name="normb", bufs=2))
        outsb = ctx.enter_context(tc.tile_pool(name="outsb", bufs=2))
        # PSUM: 8 banks total.
        #   psB x3: proj groups c0-2, then attention scores
        #   psO x2: proj groups c3-4, then attention outT accumulators
        #   psM x1: proj group c5, then rowsums
        #   psW x2: proj v-transposes, then wo accumulators
        psB = ctx.enter_context(tc.tile_pool(name="psB", bufs=2, space="PSUM"))
        psO = ctx.enter_context(tc.tile_pool(name="psO", bufs=2, space="PSUM"))
        psM = ctx.enter_context(tc.tile_pool(name="psM", bufs=1, space="PSUM"))
        psW = ctx.enter_context(tc.tile_pool(name="psW", bufs=3, space="PSUM"))

        ones_t = const.tile([P, 1], act_dt)
        nc.vector.memset(ones_t[:], 1.0)
        ident = const.tile([P, P], F32)
        make_identity(nc, ident[:])
        zbias = const.tile([P, 1], F32)
        nc.vector.memset(zbias[:], 0.0)


        # weights: resident tiles. c=0 first so the first matmuls' inputs land
        # early; the rest follow the first hsT chunk in queue order.
        w_all = wpool.tile([P, NCB, NKT * P], act_dt)
        wo_all = wpool.tile([P, HL, NHC * WOC], act_dt)

        # proj seq pieces (chunk 0 consumed k-incrementally below)
        PIECES = [(i * CW, CW) for i in range(NCH)]
        hst_tiles = {}
        _hsT_r = hsT_d.rearrange("g h p (kp s) -> g h p kp s", kp=KP, s=CW)

        def _in_dma(dst, src):
            nc.sync.dma_start(dst, src)

        def _load_hst_pack(p, g, split=False):
            t = hst.tile([P, KP, CW], act_dt, tag="hst")
            if split:  # halve the first matmul's critical transfer
                _in_dma(t[:, :KP // 2, :], _hsT_r[g, p, :, :KP // 2])
                _in_dma(t[:, KP // 2:, :], _hsT_r[g, p, :, KP // 2:])
            else:
                _in_dma(t[:, :, :], _hsT_r[g, p])
            hst_tiles.setdefault(p, []).append(t)

        def _load_w_block(c, lo, hi):  # contiguous k-tile range, wide lines
            _in_dma(w_all[:, c, lo * P:hi * P],
                    wqkv_d[c, :, lo * P:hi * P])

        # startup order: the first matmul needs only (w c0 k0-3, pack 0);
        # then alternate packs and whole weight blocks by need time
        _load_w_block(0, 0, KP)
        _load_hst_pack(0, 0)
        _load_w_block(0, KP, NKT)
        _load_hst_pack(0, 1)
        _load_w_block(1, 0, NKT)
        _load_hst_pack(0, 2)
        _load_w_block(2, 0, NKT)
        for g in range(3, NKT // KP):
            _load_hst_pack(0, g)
        for c in (3, 4, 5):
            _load_w_block(c, 0, NKT)
        # tables gate only RoPE (not the PE) — after the weight blocks
        cosT = tabs.tile([P, S], table_dt)
        _in_dma(cosT[:], cosT_d[:, :])
        ssinT = tabs.tile([P, S], table_dt)
        _in_dma(ssinT[:], ssinT_d[:, :])
        dmask = tabs.tile([P, CW // P, CW], act_dt)
        _in_dma(dmask[:], dmask_d[:, :, :])
        # wo weights after everything startup-critical; needed only ~200us in
        for c in range(HL):
            _in_dma(wo_all[:, c, :], wo_d[c].rearrange("p a b -> p (a b)"))

        # resident activations: qT (4 heads) + kT in one tile; v natural
        qkT = res.tile([P, HL + 1, S], act_dt)
        v_sb = res.tile([P, NSQ, P], act_dt)

        # ---- phase 1: projections + RoPE + v transpose ----
        def _finish_block(p, c, ps):
            off, width = PIECES[p]
            osl = bass.ds(off, width)
            if c < HL + 1:  # q heads and k: RoPE then store
                raw = evq.tile([P, CW], act_dt, tag="raw")
                nc.scalar.copy(raw[:, :width], ps[:, :width])
                t1 = rope.tile([P, CW], act_dt, tag="t1")
                nc.vector.tensor_mul(t1[:, :width], raw[:, :width], cosT[:, osl])
                t2 = rope.tile([P, CW], act_dt, tag="t2")
                nc.vector.stream_shuffle(t2[:, :width], raw[:, :width], SWAP_MASK)
                t3 = rope.tile([P, CW], act_dt, tag="t3")
                nc.vector.tensor_mul(t3[:, :width], t2[:, :width], ssinT[:, osl])
                nc.vector.tensor_add(qkT[:, c, osl], t1[:, :width], t3[:, :width])
            else:  # v: evict then PE-transpose into natural layout
                vt = vtmp.tile([P, CW], F32, tag="vt")
                nc.scalar.copy(vt[:, :width], ps[:, :width])
                for b in range(width // P):
                    pt = psW.tile([P, P], F32, tag="pf")
                    nc.tensor.transpose(pt[:], vt[:, b * P:(b + 1) * P], ident[:])
                    nc.vector.tensor_copy(v_sb[:, off // P + b, :], pt[:])

        def _load_hst_piece(p):
            for g in range(NKT // KP):
                _load_hst_pack(p, g)

        def _compute_piece_kinc(p):
            # chunk 0 runs k-incrementally in two passes of 3 column blocks
            # (3 open PSUM groups each): pass 1 starts as soon as the first
            # weight/activation slices land; pass 2 reuses the SBUF-resident
            # hsT packs.
            off, width = PIECES[p]
            packs = hst_tiles.pop(p)
            ps0 = []
            for c in range(NCB):
                pool_c = (psB, psB, psW, psO, psO, psM)[c]
                tag_c = ("ps", "ps", "pf", "po", "po", "pr")[c]
                pc = pool_c.tile([P, CW], F32, tag=tag_c, name=f"ps0_{p}_{c}")
                ps0.append(pc)
            for cs in ((0, 1, 2), (3, 4, 5)):
                for k in range(NKT):
                    for c in cs:
                        nc.tensor.matmul(
                            ps0[c][:, :width], w_all[:, c, k * P:(k + 1) * P],
                            packs[k // KP][:, k % KP, :width],
                            start=(k == 0), stop=(k == NKT - 1))
            return ps0

        ps_p0 = _compute_piece_kinc(0)
        _load_hst_piece(1)
        for c in range(NCB):
            _finish_block(0, c, ps_p0[c])

        for p in range(1, len(PIECES)):
            if p + 1 < len(PIECES):
                _load_hst_piece(p + 1)
            packs = hst_tiles.pop(p)
            width = PIECES[p][1]
            for c in range(NCB):
                ps = psB.tile([P, CW], F32, tag="ps")
                for k in range(NKT):
                    nc.tensor.matmul(
                        ps[:, :width], w_all[:, c, k * P:(k + 1) * P],
                        packs[k // KP][:, k % KP, :width],
                        start=(k == 0), stop=(k == NKT - 1))
                _finish_block(p, c, ps)

        # ---- phase 2+3 interleaved: attention chunk j, then its wo ----
        for j in range(NCH):
            jsl = bass.ts(j, CW)
            nd = CW // P              # 4 diagonal blocks
            nblk = (j + 1) * nd
            d0 = j * nd               # first diagonal sk block
            # Block order puts each pair adjacent so a prob tile only lives
            # ~2 blocks: (t0,t1), then each remaining diag paired with a full
            # block, then the remaining fulls pairwise. Pair merging happens
            # in-place (bf16 add into the wider tile) on the vector engine;
            # ONE rowsum matmul per pair replaces one per block.
            if j == 0:
                order = [d0, d0 + 1, d0 + 2, d0 + 3]
                pairs = [(0, 1, 128, 512), (2, 3, 384, 512)]
                rows = [(0, 0, 512), (2, 256, 512)]
            else:
                order = [d0, d0 + 1, 0, d0 + 2, 1, d0 + 3] + list(range(2, 4 * j))
                pairs = [(0, 1, 128, 512), (2, 3, 256, 512), (4, 5, 384, 512)]
                pairs += [(6 + 2 * i, 7 + 2 * i, 0, 512) for i in range(2 * j - 1)]
                rows = [(a, 0, 512) for a, _, _, _ in pairs]
            outT = outc.tile([P, HL, CW], act_dt, tag="oc")
            for h in range(HL):
                po = psO.tile([P, CW], F32, tag="po")
                pr = psM.tile([1, CW], F32, tag="pr")
                pbs = {}
                nrow = len(rows)

                def _score_exp(bi):
                    sk = order[bi]
                    t = sk - d0
                    o = t * P if 0 <= t else 0
                    csl = bass.ds(j * CW + o, CW - o)
                    pss = psB.tile([P, CW], F32, tag="ps")
                    nc.tensor.matmul(
                        pss[:, o:], qkT[:, HL, sk * P:(sk + 1) * P],
                        qkT[:, h, csl], start=True, stop=True)
                    pb = probs.tile([P, CW], act_dt, tag="pb")
                    nc.scalar.activation(
                        pb[:, o:], pss[:, o:], mybir.ActivationFunctionType.Exp,
                        scale=SCALE)
                    if 0 <= t:  # diagonal block: zero out sq < sk entries
                        nc.vector.tensor_mul(pb[:, o:], pb[:, o:], dmask[:, t, o:])
                    pbs[bi] = pb
                    return sk, o, pb

                def _pv(bi, sk, o, pb):
                    nc.tensor.matmul(po[:, o:], v_sb[:, sk, :], pb[:, o:],
                                     start=(bi == 0), stop=(bi == nblk - 1))

                # emit per pair: both scores, then both PVs back-to-back so
                # the po-bank accumulates run consecutively
                for a, b, lo, hi in pairs:
                    ra = _score_exp(a)
                    rb = _score_exp(b)
                    _pv(a, *ra)
                    _pv(b, *rb)
                    nc.vector.tensor_add(
                        pbs[a][:, lo:hi], pbs[a][:, lo:hi], pbs[b][:, lo:hi])
                # binary-tree merge of pair-survivors on the DVE until one
                # tile remains -> a single rowsum matmul per (head, chunk)
                rows2 = list(rows)
                while len(rows2) > 1:
                    nxt = []
                    i = 0
                    while i < len(rows2):
                        if i + 1 < len(rows2):
                            a0, l0, h0 = rows2[i]
                            a1, l1, h1 = rows2[i + 1]
                            nc.vector.tensor_add(
                                pbs[a0][:, l1:h1], pbs[a0][:, l1:h1],
                                pbs[a1][:, l1:h1])
                            nxt.append((a0, min(l0, l1), max(h0, h1)))
                            i += 2
                        else:
                            nxt.append(rows2[i])
                            i += 1
                    rows2 = nxt
                a, rlo, rhi = rows2[0]
                nc.tensor.matmul(pr[:, rlo:rhi], ones_t[:], pbs[a][:, rlo:rhi],
                                 start=True, stop=True)
                rc = norm.tile([1, CW], F32, tag="rc")
                nc.vector.reciprocal_approx_fast(rc[:], pr[:])
                rb = normb.tile([P, CW], F32, tag="rb")
                nc.gpsimd.partition_broadcast(rb[:], rc[:], channels=P)
                nc.vector.tensor_mul(outT[:, h, :], po[:], rb[:])

            # wo projection for this chunk (weights resident); bf16 partials.
            # Low priority: wo work fills engine gaps during the next chunk's
            # attention instead of blocking its exps/adds in the ready heaps.
            with tc.high_priority(offset=-(1 << 20)):
                gw = 4
                for sq in range(CW // P):
                    row = bass.ds((j * (CW // P) + sq) * P, P)
                    for hp in range(NHC // gw):
                        ob = outsb.tile([P, 4 * WOC], act_dt, tag="ob")
                        for half in range(gw):
                            hc = gw * hp + half
                            pf = psW.tile([P, WOC], F32, tag="pf")
                            for c in range(HL):
                                nc.tensor.matmul(
                                    pf[:], outT[:, c, sq * P:(sq + 1) * P],
                                    wo_all[:, c, hc * WOC:(hc + 1) * WOC],
                                    start=(c == 0), stop=(c == HL - 1))
                            oslc = bass.ds(half * WOC, WOC)
                            if half % 2 == 0:
                                nc.vector.tensor_copy(ob[:, oslc], pf[:])
                            else:
                                nc.scalar.copy(ob[:, oslc], pf[:])
                        eng = nc.sync if hp % 2 == 0 else nc.gpsimd
                        eng.dma_start(
                            out_d[row, bass.ds(hp * gw * WOC, gw * WOC)],
                            ob[:, :gw * WOC])

    nc.compile()
    return nc


def _rope_tables():
    inv_freq = (1.0 / (BASE ** (np.arange(0, D, 2, dtype=np.float32) / D))).astype(np.float32)
    pos = np.arange(S, dtype=np.float32)[:, None]
    ang = pos * inv_freq[None, :]              # [S, D/2]
    sin = np.sin(ang).astype(np.float32).T     # [D/2, S]
    cos = np.cos(ang).astype(np.float32).T
    cosT = np.empty((D, S), np.float32)
    cosT[0::2] = cos
    cosT[1::2] = cos
    ssinT = np.empty((D, S), np.float32)
    ssinT[0::2] = -sin
    ssinT[1::2] = sin
    return cosT, ssinT


def _diag_masks():
    # dmask[p, t, f] = 1 where the scoreT element (sk=128t+p, sq=f) is causal-valid
    p = np.arange(P)[:, None, None]
    t = np.arange(CW // P)[None, :, None]
    f = np.arange(CW)[None, None, :]
    return (f >= P * t + p).astype(np.float32)


_NC_CACHE = {}


def kernel(hidden_states, wq, wk, wv, wo):
    global LAST_RESULT
    act_np = ml_dtypes.bfloat16
    key = "bf16"
    if key not in _NC_CACHE:
        _NC_CACHE[key] = _build()
    nc = _NC_CACHE[key]

    hs = np.asarray(hidden_states, np.float32).reshape(S, HID)
    hsT = np.ascontiguousarray(hs.T).astype(act_np)
    hsT = np.ascontiguousarray(
        hsT.reshape(NKT // KP, KP, P, NCH, CW).transpose(0, 3, 2, 1, 4)
        .reshape(NKT // KP, NCH, P, KP * CW))
    cosT, ssinT = _rope_tables()
    dmask = _diag_masks().astype(act_np)

    in_maps = []
    for i in range(NCORES):
        wqkv = np.concatenate(
            [np.asarray(wq, np.float32)[:, i * HL * D:(i + 1) * HL * D],
             np.asarray(wk, np.float32)[:, i * D:(i + 1) * D],
             np.asarray(wv, np.float32)[:, i * D:(i + 1) * D]], axis=1)
        # [HID, 768] -> [NCB, P, NKT*P]: block c, hid-in-tile p, (k-tile, col)
        wqkv = np.ascontiguousarray(
            wqkv.reshape(NKT, P, NCB, P).transpose(2, 1, 0, 3).reshape(NCB, P, NKT * P)
        ).astype(act_np)
        wo_i = np.ascontiguousarray(
            np.asarray(wo, np.float32)[i * HL * D:(i + 1) * HL * D, :]
            .reshape(HL, P, NHC, WOC)).astype(act_np)
        in_maps.append({
            "hsT": hsT, "wqkv": wqkv, "wo": wo_i,
            "cosT": cosT, "ssinT": ssinT, "dmask": dmask,
        })

    trace = bool(os.environ.get("BASS_KERNEL_TRACE"))
    res = run_bass_kernel_spmd(nc, in_maps, list(range(NCORES)),
                               trace=trace, trace_cores=[0] if trace else None)
    LAST_RESULT = res
    acc = np.zeros((S, HID), np.float32)
    for i in range(NCORES):
        acc += np.asarray(res.results[i]["out"], np.float32)
    return acc.reshape(B, S, HID)
